# revision 12
# baseline (speedup 1.0000x reference)
"""DSA varlen sparse attention on 8 Trainium2 cores.

Math: the reference's softmax -> *topk_scores -> renormalize collapses to
    out[t,h,:] = sum_k exp(s_k)*ts_k*v_k / sum_k exp(s_k)*ts_k
(the first softmax's denominator cancels; the +1e-12 is ~2e-12 relative).
Folding the (head-shared, clamped, possibly duplicated) top-k indices into a
dense mask b[t,s] = sum_k ts[t,k]*[lidx[t,k]==s] over the owning document
turns this into dense attention over the doc with a multiplicative mask:
    out[t,h,:] = sum_s exp(score[t,h,s]) * b[t,s] * v[s,h,:] / (same with v=1)

The mask is applied additively in log space: ln(b) is accumulated into the
score PSUM tile with an identity matmul before the exp.

Sharding: tokens across 8 cores (512 tokens each, half a document); each core
keeps its whole document's K/V resident, so all top-k targets are local.
The host wrapper does the sharding plus the index densification (np.add.at
of topk_scores into the per-core mask); all FLOP-carrying compute (score and
output matmuls, exp/ln, normalization) runs on device.
"""

import numpy as np

import concourse.bacc as bacc
import concourse.mybir as mybir
import concourse.tile as tile
from concourse.bass_utils import run_bass_kernel_spmd
from concourse.masks import make_identity

F32 = mybir.dt.float32
AF = mybir.ActivationFunctionType
OP = mybir.AluOpType

# problem geometry (hardcoded per spec)
T, H, D, K, NUM_DOCS, NCORES = 4096, 16, 64, 64, 4, 8
TLOC = T // NCORES      # 512 query tokens per core
S = T // NUM_DOCS       # 1024 keys (one document) per core
P = 128
TC = TLOC // P          # 4 token chunks
SC = S // P             # 8 key chunks
HD = H * D              # 1024
SCALE = float(D) ** -0.5
B_EPS = 1e-20           # mask background (within ACT Ln spline range)


def build_program():
    nc = bacc.Bacc("TRN2", target_bir_lowering=False, debug=False)

    q_d = nc.dram_tensor("q", [TLOC, HD], F32, kind="ExternalInput")
    k_d = nc.dram_tensor("k", [S, HD], F32, kind="ExternalInput")
    v_d = nc.dram_tensor("v", [S, HD], F32, kind="ExternalInput")
    bT_d = nc.dram_tensor("bT", [S, TLOC], F32, kind="ExternalInput")
    out_d = nc.dram_tensor("out", [H, D, TLOC], F32, kind="ExternalOutput")

    with tile.TileContext(nc) as tc:
        with (
            tc.tile_pool(name="const", bufs=1) as constp,
            tc.tile_pool(name="qn", bufs=2) as qnp,
            tc.tile_pool(name="kn", bufs=2) as knp,
            tc.tile_pool(name="vn", bufs=2) as vnp,
            tc.tile_pool(name="vaug", bufs=SC) as vaugp,
            tc.tile_pool(name="qT", bufs=H // 2) as qTp,
            tc.tile_pool(name="kT", bufs=H // 2) as kTp,
            tc.tile_pool(name="lnb", bufs=SC) as lnbp,
            tc.tile_pool(name="w", bufs=3) as wp,
            tc.tile_pool(name="fin", bufs=3) as finp,
            tc.tile_pool(name="dscr", bufs=3, space="DRAM") as dscrp,
        ):
            # ---- constants -------------------------------------------------
            ident = constp.tile([P, P], F32)
            make_identity(nc, ident[:])

            # ---- bulk input loads (start DMAs early) -----------------------
            q_sb, k_sb, v_sb = [], [], []
            for c in range(TC):
                t_ = qnp.tile([P, HD], F32, tag="qn")
                nc.sync.dma_start(out=t_[:], in_=q_d[c * P:(c + 1) * P, :])
                q_sb.append(t_)
            for c in range(SC):
                t_ = knp.tile([P, HD], F32, tag="kn")
                nc.sync.dma_start(out=t_[:], in_=k_d[c * P:(c + 1) * P, :])
                k_sb.append(t_)
            for c in range(SC):
                t_ = vnp.tile([P, HD], F32, tag="vn")
                nc.sync.dma_start(out=t_[:], in_=v_d[c * P:(c + 1) * P, :])
                v_sb.append(t_)

            # mask -> ln(mask), all Ln ops up front (one ACT table set with Exp)
            lnb_sb = []
            for c in range(SC):
                raw = finp.tile([P, TLOC], F32, tag="braw")
                nc.sync.dma_start(out=raw[:], in_=bT_d[c * P:(c + 1) * P, :])
                t_ = lnbp.tile([P, TLOC], F32, tag="lnb")
                nc.scalar.activation(t_[:], raw[:], AF.Ln)
                lnb_sb.append(t_)

            # ---- transposes: qT (d,t) and kT (d,s), two heads per tile -----
            with tc.tile_pool(name="ptr", bufs=2, space="PSUM") as ptrp:
                qT_sb = []
                for hp in range(H // 2):
                    t_ = qTp.tile([P, TLOC], F32, tag="qT")
                    qT_sb.append(t_)
                for c in range(TC):
                    for hp in range(H // 2):
                        pt = ptrp.tile([P, P], F32, tag="ptr")
                        nc.tensor.transpose(
                            pt[:], q_sb[c][:, hp * P:(hp + 1) * P], ident[:]
                        )
                        nc.vector.tensor_copy(
                            qT_sb[hp][:, c * P:(c + 1) * P], pt[:]
                        )
                kT_sb = []
                for hp in range(H // 2):
                    t_ = kTp.tile([P, S], F32, tag="kT")
                    kT_sb.append(t_)
                for c in range(SC):
                    for hp in range(H // 2):
                        pt = ptrp.tile([P, P], F32, tag="ptr")
                        nc.tensor.transpose(
                            pt[:], k_sb[c][:, hp * P:(hp + 1) * P], ident[:]
                        )
                        # fold in the attention scale on the drain
                        nc.scalar.mul(
                            kT_sb[hp][:, c * P:(c + 1) * P], pt[:], SCALE
                        )

            # ---- v_aug: per head [v | ones]; the ones column makes the
            # out-matmul emit the softmax denominator in psum row D ----------
            v_aug = []
            for c in range(SC):
                t_ = vaugp.tile([P, H * (D + 1)], F32, tag="vaug")
                nc.gpsimd.memset(t_[:], 1.0)
                nc.gpsimd.tensor_copy(
                    t_[:].rearrange("p (h e) -> p h e", h=H)[:, :, 0:D],
                    v_sb[c][:].rearrange("p (h d) -> p h d", h=H),
                )
                v_aug.append(t_)

            # ---- main head loop -------------------------------------------
            with (
                tc.tile_pool(name="psc", bufs=3, space="PSUM") as pscp,
                tc.tile_pool(name="pout", bufs=2, space="PSUM") as poutp,
            ):
                for h in range(H):
                    hp, hh = h // 2, h % 2
                    kT_h = kT_sb[hp]
                    qT_h = qT_sb[hp][64 * hh:64 * (hh + 1), :]
                    pout = poutp.tile([D + 1, TLOC], F32, tag="pout")
                    for scp in range(SC // 2):
                        psc = pscp.tile([P, 2 * TLOC], F32, tag="psc")
                        w_t = wp.tile([P, 2 * TLOC], F32, tag="w")
                        for half in range(2):
                            s_c = 2 * scp + half
                            sl = slice(half * TLOC, (half + 1) * TLOC)
                            # scores^T (s,t) accumulated with ln(b)^T
                            nc.tensor.matmul(
                                psc[:, sl],
                                lhsT=kT_h[64 * hh:64 * (hh + 1),
                                          s_c * P:(s_c + 1) * P],
                                rhs=qT_h,
                                start=True, stop=False,
                            )
                            nc.tensor.matmul(
                                psc[:, sl],
                                lhsT=ident[:],
                                rhs=lnb_sb[s_c][:],
                                start=False, stop=True,
                            )
                        nc.scalar.activation(w_t[:], psc[:], AF.Exp)
                        for half in range(2):
                            s_c = 2 * scp + half
                            sl = slice(half * TLOC, (half + 1) * TLOC)
                            nc.tensor.matmul(
                                pout[:],
                                lhsT=v_aug[s_c][:, h * (D + 1):(h + 1) * (D + 1)],
                                rhs=w_t[:, sl],
                                start=(s_c == 0), stop=(s_c == SC - 1),
                            )
                    # normalize: row D of pout is the denominator.
                    # partition_broadcast ucode misreads non-zero base
                    # partitions on HW, so broadcast via a DRAM bounce.
                    rn = finp.tile([P, TLOC], F32, tag="rn")
                    nc.vector.reciprocal(rn[D:D + 1, :], pout[D:D + 1, :])
                    rnd = dscrp.tile([1, TLOC], F32, tag="rnd")
                    nc.sync.dma_start(out=rnd[:], in_=rn[D:D + 1, :])
                    rnb = finp.tile([D, TLOC], F32, tag="rnb")
                    nc.sync.dma_start(
                        out=rnb[:], in_=rnd[:].to_broadcast([D, TLOC]))
                    outn = finp.tile([D, TLOC], F32, tag="outn")
                    nc.vector.tensor_tensor(
                        out=outn[:], in0=pout[0:D, :], in1=rnb[:], op=OP.mult,
                    )
                    nc.sync.dma_start(out=out_d[h], in_=outn[:])

    nc.compile()
    return nc


_PROG = None


def _get_program():
    global _PROG
    if _PROG is None:
        _PROG = build_program()
    return _PROG


def make_in_maps(q, k, v, idx, ts):
    """Shard across cores + densify the top-k selection into per-core masks."""
    in_maps = []
    tloc_ar = np.arange(TLOC)
    for c in range(NCORES):
        doc = c // 2
        lidx = np.clip(idx[c * TLOC:(c + 1) * TLOC] - doc * S, 0, S - 1)
        b = np.full((TLOC, S), B_EPS, dtype=np.float64)
        np.add.at(b, (tloc_ar[:, None], lidx), ts[c * TLOC:(c + 1) * TLOC])
        in_maps.append({
            "q": np.ascontiguousarray(
                q[c * TLOC:(c + 1) * TLOC].reshape(TLOC, HD), dtype=np.float32),
            "k": np.ascontiguousarray(
                k[doc * S:(doc + 1) * S].reshape(S, HD), dtype=np.float32),
            "v": np.ascontiguousarray(
                v[doc * S:(doc + 1) * S].reshape(S, HD), dtype=np.float32),
            "bT": np.ascontiguousarray(b.T, dtype=np.float32),
        })
    return in_maps


def assemble(results):
    out = np.empty((T, H, D), dtype=np.float32)
    for c in range(NCORES):
        out[c * TLOC:(c + 1) * TLOC] = np.transpose(
            results[c]["out"], (2, 0, 1))
    return out


def kernel(q_packed, k_packed, v_packed, cu_seqlens, topk_indices, topk_scores):
    q = np.asarray(q_packed, dtype=np.float32)
    k = np.asarray(k_packed, dtype=np.float32)
    v = np.asarray(v_packed, dtype=np.float32)
    idx = np.asarray(topk_indices, dtype=np.int32)
    ts = np.asarray(topk_scores, dtype=np.float32)
    nc = _get_program()
    in_maps = make_in_maps(q, k, v, idx, ts)
    res = run_bass_kernel_spmd(nc, in_maps, core_ids=list(range(NCORES)))
    return assemble(res.results)
